# revision 7
# baseline (speedup 1.0000x reference)
"""ArcFace loss kernel for 8 Trainium2 NeuronCores.

Model-parallel over the identities axis (I=100000 -> 12500 per core):
  pass 1: local sum(w^2) over identities  -> AllGather -> inv norms
  pass 2: logits = 64*cos(theta + margin*onehot) via bf16 matmuls,
          online sum(exp(logit - 20)) per row, logits stashed to DRAM
  AllGather row sums -> logsumexp
  pass 3: out = logits - logsumexp
"""

import math
import sys

if "/opt/trn_rl_repo" not in sys.path:
    sys.path.insert(0, "/opt/trn_rl_repo")

import numpy as np

import concourse.mybir as mybir
from concourse import bacc, tile
from concourse.alu_op_type import AluOpType
from concourse.bass_utils import run_bass_kernel_spmd

NCORES = 8
B, E, I, S = 512, 512, 100000, 3
IL = I // NCORES      # identities per core
IT = 500              # identities per matmul tile
NIT = IL // IT        # 25 i-tiles
G = 2                 # i-tiles per staging group
NG = (NIT + G - 1) // G
BC = B // 128         # batch chunks of 128
EC = E // 128         # embedding chunks of 128

MARGIN = 0.5
SCALE = 64.0
C0 = 20.0                           # fixed exp shift (|logit| <= ~25 for this data)
K1_64 = 1.0 - math.cos(MARGIN)      # (SCALE*(1-cos m))/SCALE
K2 = SCALE * math.sin(MARGIN)
EPS = 1e-12

F32 = mybir.dt.float32
BF16 = mybir.dt.bfloat16
X = mybir.AxisListType.X

_cache = {}


def _build():
    nc = bacc.Bacc("TRN2", target_bir_lowering=False, debug=False,
                   num_devices=NCORES)
    wt = nc.dram_tensor("wt", [S * E, IL], F32, kind="ExternalInput").ap()
    embT = nc.dram_tensor("embT", [E, B], F32, kind="ExternalInput").ap()
    tgt = nc.dram_tensor("tgt", [B, IL], F32, kind="ExternalInput").ap()
    out = nc.dram_tensor("out", [B, IL], F32, kind="ExternalOutput").ap()

    rg = [list(range(NCORES))]

    with tile.TileContext(nc) as tc:
        from contextlib import ExitStack
        with ExitStack() as st:
            p_const = st.enter_context(tc.tile_pool(name="const", bufs=1))
            p_sq = st.enter_context(tc.tile_pool(name="sq", bufs=2))
            p_w2 = st.enter_context(tc.tile_pool(name="w2", bufs=3))
            p_t = st.enter_context(tc.tile_pool(name="tp", bufs=2))
            p_m64 = st.enter_context(tc.tile_pool(name="m64", bufs=2))
            p_work = st.enter_context(tc.tile_pool(name="work", bufs=2))
            p_p3 = st.enter_context(tc.tile_pool(name="p3", bufs=2))
            p_psum = st.enter_context(tc.tile_pool(name="ps", bufs=6, space="PSUM"))
            p_dram = st.enter_context(tc.tile_pool(name="dram", bufs=1, space="DRAM"))

            # bias constants for activations (float bias needs a const AP)
            bias_k22 = p_const.tile([128, 1], F32)
            nc.vector.memset(bias_k22[:], K2 * K2)
            bias_nc0 = p_const.tile([128, 1], F32)
            nc.vector.memset(bias_nc0[:], -C0)

            # ---------------- pass 1: sum of squares over local identities
            s2parts = p_const.tile([128, S * EC * NIT], F32)
            for it in range(NIT):
                for s in range(S):
                    w1 = p_w2.tile([128, EC, IT], BF16, name=f"ws{s}")
                    src = wt[s * E:(s + 1) * E, it * IT:(it + 1) * IT] \
                        .rearrange("(c p) i -> p c i", p=128)
                    nc.gpsimd.dma_start(w1[:], src)
                    sqs = p_sq.tile([128, IT], BF16, name="sqs")
                    for c in range(EC):
                        col = (s * EC + c) * NIT + it
                        nc.scalar.activation(
                            sqs[:], w1[:, c, :],
                            mybir.ActivationFunctionType.Square,
                            accum_out=s2parts[:, col:col + 1])

            sumsq = p_const.tile([128, S * EC], F32)
            for j in range(S * EC):
                nc.vector.tensor_reduce(
                    sumsq[:, j:j + 1], s2parts[:, j * NIT:(j + 1) * NIT],
                    X, AluOpType.add)

            # ---------------- allgather partial sums, compute 64/norm
            ag1_in = p_dram.tile([128, S * EC], F32)
            ag1_out = p_dram.tile([128 * NCORES, S * EC], F32)
            nc.sync.dma_start(ag1_in[:], sumsq[:])
            nc.gpsimd.collective_compute(
                "AllGather", AluOpType.bypass, replica_groups=rg,
                ins=[ag1_in.opt()], outs=[ag1_out.opt()])
            agb1 = p_const.tile([128, S * EC, NCORES], F32)
            for r in range(NCORES):
                nc.sync.dma_start(agb1[:, :, r], ag1_out[r * 128:(r + 1) * 128, :])
            gss = p_const.tile([128, S * EC], F32)
            nc.vector.tensor_reduce(gss[:], agb1[:], X, AluOpType.add)

            norm = p_const.tile([128, S * EC], F32)
            nc.scalar.activation(norm[:], gss[:],
                                 mybir.ActivationFunctionType.Sqrt)
            nc.vector.tensor_scalar_max(norm[:], norm[:], EPS)
            inv = p_const.tile([128, S * EC], F32)
            nc.vector.reciprocal(inv[:], norm[:])
            # one newton step: inv = inv*(2 - norm*inv)
            nt = p_const.tile([128, S * EC], F32)
            nc.vector.scalar_tensor_tensor(nt[:], norm[:], 0.0, inv[:],
                                           AluOpType.bypass, AluOpType.mult)
            nc.vector.tensor_scalar(nt[:], nt[:], -1.0, 2.0,
                                    AluOpType.mult, AluOpType.add)
            nc.vector.scalar_tensor_tensor(inv[:], inv[:], 0.0, nt[:],
                                           AluOpType.bypass, AluOpType.mult)

            # ---------------- scaled transposed embeddings, bf16
            embT_sb = p_const.tile([128, EC, B], F32)
            nc.sync.dma_start(embT_sb[:], embT.rearrange("(c p) b -> p c b", p=128))
            embS = []
            for s in range(S):
                es = p_const.tile([128, EC, B], BF16, name=f"embS{s}")
                for c in range(EC):
                    nc.vector.tensor_scalar(
                        es[:, c, :], embT_sb[:, c, :],
                        inv[:, s * EC + c:s * EC + c + 1], SCALE,
                        AluOpType.mult, AluOpType.mult)
                embS.append(es)

            # ---------------- pass 2: matmuls, margin, exp-sums, stash
            stash = p_dram.tile([B, IL], F32)
            sexp_parts = p_const.tile([128, BC * NG], F32)
            for g in range(NG):
                its = list(range(g * G, min(NIT, (g + 1) * G)))
                W = len(its) * IT
                i0 = its[0] * IT
                m64 = p_m64.tile([128, BC, G * IT], F32, name="m64")
                work = p_work.tile([128, BC, G * IT], F32, name="work")
                ttile = p_t.tile([128, BC, G * IT], BF16, name="ttile")
                nc.gpsimd.dma_start(
                    ttile[:, :, :W],
                    tgt[:, i0:i0 + W].rearrange("(b p) i -> p b i", p=128))
                wtl = []
                for k, it in enumerate(its):
                    wsl = []
                    for s in range(S):
                        ws = p_w2.tile([128, EC, IT], BF16, name=f"ws{s}")
                        src = wt[s * E:(s + 1) * E, it * IT:(it + 1) * IT] \
                            .rearrange("(c p) i -> p c i", p=128)
                        nc.gpsimd.dma_start(ws[:], src)
                        wsl.append(ws)
                    wtl.append(wsl)
                for k, it in enumerate(its):
                    off = k * IT
                    for b in range(BC):
                        pss = []
                        for s in range(S):
                            ps = p_psum.tile([128, IT], F32, name="ps")
                            for c in range(EC):
                                nc.tensor.matmul(
                                    ps[:],
                                    embS[s][:, c, b * 128:(b + 1) * 128],
                                    wtl[k][s][:, c, :],
                                    start=(c == 0), stop=(c == EC - 1))
                            pss.append(ps)
                        dst = m64[:, b, off:off + IT]
                        nc.scalar.activation(dst, pss[0][:],
                                             mybir.ActivationFunctionType.Copy)
                        nc.vector.scalar_tensor_tensor(
                            dst, pss[1][:], 0.0, dst,
                            AluOpType.bypass, AluOpType.max)
                        nc.vector.scalar_tensor_tensor(
                            dst, pss[2][:], 0.0, dst,
                            AluOpType.bypass, AluOpType.max)
                # m64 holds 64*cos. work = sqrt(K2^2 - (K2/64)^2 * m64^2) = K2*sin
                nc.scalar.activation(work[:, :, :W], m64[:, :, :W],
                                     mybir.ActivationFunctionType.Square,
                                     scale=1.0 / SCALE)
                nc.scalar.activation(work[:, :, :W], work[:, :, :W],
                                     mybir.ActivationFunctionType.Sqrt,
                                     bias=bias_k22[:], scale=-(K2 * K2))
                # work = K1/64 * m64 + K2*sin(theta)
                nc.vector.scalar_tensor_tensor(
                    work[:, :, :W], m64[:, :, :W], K1_64, work[:, :, :W],
                    AluOpType.mult, AluOpType.add)
                # work = -(work) * target
                nc.vector.scalar_tensor_tensor(
                    work[:, :, :W], work[:, :, :W], -1.0, ttile[:, :, :W],
                    AluOpType.mult, AluOpType.mult)
                # logits (into m64) = m64 + work
                nc.vector.scalar_tensor_tensor(
                    m64[:, :, :W], work[:, :, :W], 0.0, m64[:, :, :W],
                    AluOpType.bypass, AluOpType.add)
                # per-b exp(logits - C0), accumulate row sums
                for b in range(BC):
                    nc.scalar.activation(
                        work[:, b, :W], m64[:, b, :W],
                        mybir.ActivationFunctionType.Exp, bias=bias_nc0[:],
                        accum_out=sexp_parts[:, b * NG + g:b * NG + g + 1])
                nc.sync.dma_start(
                    stash[:, i0:i0 + W].rearrange("(b p) i -> p b i", p=128),
                    m64[:, :, :W])

            # ---------------- allgather row sums -> logsumexp
            sloc = p_const.tile([128, BC], F32)
            for b in range(BC):
                nc.vector.tensor_reduce(
                    sloc[:, b:b + 1], sexp_parts[:, b * NG:(b + 1) * NG],
                    X, AluOpType.add)
            ag2_in = p_dram.tile([128, BC], F32)
            ag2_out = p_dram.tile([128 * NCORES, BC], F32)
            nc.sync.dma_start(ag2_in[:], sloc[:])
            nc.gpsimd.collective_compute(
                "AllGather", AluOpType.bypass, replica_groups=rg,
                ins=[ag2_in.opt()], outs=[ag2_out.opt()])
            agb2 = p_const.tile([128, BC, NCORES], F32)
            for r in range(NCORES):
                nc.sync.dma_start(agb2[:, :, r], ag2_out[r * 128:(r + 1) * 128, :])
            sg = p_const.tile([128, BC], F32)
            nc.vector.tensor_reduce(sg[:], agb2[:], X, AluOpType.add)
            lse = p_const.tile([128, BC], F32)
            nc.scalar.activation(lse[:], sg[:], mybir.ActivationFunctionType.Ln)

            # ---------------- pass 3: out = logits - lse - C0
            J = 2500
            for b in range(BC):
                for j in range(IL // J):
                    lt = p_p3.tile([128, J], F32, name="lt")
                    nc.sync.dma_start(
                        lt[:], stash[b * 128:(b + 1) * 128, j * J:(j + 1) * J])
                    nc.vector.tensor_scalar(
                        lt[:], lt[:], lse[:, b:b + 1], C0,
                        AluOpType.subtract, AluOpType.subtract)
                    nc.sync.dma_start(
                        out[b * 128:(b + 1) * 128, j * J:(j + 1) * J], lt[:])

    nc.compile()
    return nc


def _get_nc():
    if "nc" not in _cache:
        _cache["nc"] = _build()
    return _cache["nc"]


def _shard(embedding_batch, target_batch, w):
    embT = np.ascontiguousarray(embedding_batch.T, dtype=np.float32)
    # (E, I, S) -> (S, E, I) once, then contiguous per-core slices
    wT = np.ascontiguousarray(np.transpose(w, (2, 0, 1)), dtype=np.float32)
    in_maps = []
    for k in range(NCORES):
        lo, hi = k * IL, (k + 1) * IL
        in_maps.append({
            "wt": np.ascontiguousarray(wT[:, :, lo:hi]).reshape(S * E, IL),
            "embT": embT,
            "tgt": np.ascontiguousarray(target_batch[:, lo:hi], dtype=np.float32),
        })
    return in_maps


def run_sharded(embedding_batch, target_batch, w, trace=False, trace_kwargs=None):
    nc = _get_nc()
    in_maps = _shard(embedding_batch, target_batch, w)
    res = run_bass_kernel_spmd(nc, in_maps, core_ids=list(range(NCORES)),
                               trace=trace, **(trace_kwargs or {}))
    full = np.concatenate([res.results[k]["out"] for k in range(NCORES)], axis=1)
    return full, res


def kernel(embedding_batch, target_batch, w):
    full, _ = run_sharded(embedding_batch, target_batch, w)
    return full
